# revision 14
# baseline (speedup 1.0000x reference)
"""Transformer block (pre-LN attention + FFN) on 8 TRN2 NeuronCores.

Sharding: batch x head tensor-parallel for attention, sequence-parallel for
LN/FFN/residual. Core c (b = c//4, j = c%4):
  - owns token shard [512j : 512j+512) of batch b for LN1/LN2/FFN/residual
  - owns heads [4j : 4j+4) of batch b for attention (all 2048 tokens)
Collectives (per-batch groups [[0..3],[4..7]]):
  - AllGather of transposed LN1 output hT (bf16) so every core sees all tokens
  - ReduceScatter (add) of the attention projection partial sums back to
    token shards.
All programs are identical across cores (SPMD); only input data differs.

Matmul dtypes: float32r (full-rate fp32, no cast needed) for the
weight-side matmuls fed by f32 DRAM (FFN1), bf16 for attention internals +
QKV/proj/FFN2 (operands produced on-chip, cast free on evacuation).
"""

import numpy as np

import concourse.bass as bass
import concourse.mybir as mybir
import concourse.tile as tile
from concourse import bacc
from concourse.bass_utils import run_bass_kernel_spmd
from concourse.masks import make_identity

P = 128
C = 1024          # n_embd
KT = C // P       # 8 c-tiles
T_OWN = 512       # tokens per core (sequence shard)
IT = T_OWN // P   # 4 own t-tiles
T_ALL = 2048      # tokens per batch
NH = 4            # heads per core
D = 64            # head dim
DL = NH * D       # 256 local head features
FF = 4096
FMT = FF // P     # 32 ffn m-tiles
CH = 256          # attention chunk
QC = T_ALL // CH  # 8 chunks
EPS = 1e-5
SCALE = 1.0 / 32.0  # C ** -0.5
GROUPS = [[0, 1, 2, 3], [4, 5, 6, 7]]
NCORES = 8

f32 = mybir.dt.float32
f32r = mybir.dt.float32r
bf16 = mybir.dt.bfloat16
AX = mybir.AxisListType
ALU = mybir.AluOpType
ACT_F = mybir.ActivationFunctionType


def _layer_norm(nc, sb, x_view, w_rep, b_rep, out_view, tmp_tag):
    """Token-major LN over free axis. x_view/out_view: [P, IT, C]."""
    for i in range(IT):
        xi = x_view[:, i, :]
        tmp = sb.tile([P, C], f32, tag="outev", bufs=2, name=f"ln_tmp_{tmp_tag}_{i}")
        ssum = sb.tile([P, 1], f32, tag=tmp_tag + "s", name=f"ln_s_{tmp_tag}_{i}")
        nc.vector.tensor_reduce(ssum[:], xi, AX.X, ALU.add)
        mu = sb.tile([P, 1], f32, tag=tmp_tag + "mu", name=f"ln_mu_{tmp_tag}_{i}")
        nc.vector.tensor_scalar_mul(mu[:], ssum[:], 1.0 / C)
        nc.vector.tensor_mul(out=tmp[:], in0=xi, in1=xi)
        sqs = sb.tile([P, 1], f32, tag=tmp_tag + "q", name=f"ln_q_{tmp_tag}_{i}")
        nc.vector.tensor_reduce(sqs[:], tmp[:], AX.X, ALU.add)
        # var = sqs/C - mu^2 ; rstd = sqrt(1/(var+eps))
        var = sb.tile([P, 1], f32, tag=tmp_tag + "v", name=f"ln_v_{tmp_tag}_{i}")
        nc.vector.tensor_scalar_mul(var[:], sqs[:], 1.0 / C)
        musq = sb.tile([P, 1], f32, tag=tmp_tag + "m2", name=f"ln_m2_{tmp_tag}_{i}")
        nc.vector.tensor_mul(out=musq[:], in0=mu[:], in1=mu[:])
        nc.vector.tensor_sub(out=var[:], in0=var[:], in1=musq[:])
        nc.vector.tensor_scalar_add(var[:], var[:], EPS)
        rv = sb.tile([P, 1], f32, tag=tmp_tag + "rv", name=f"ln_rv_{tmp_tag}_{i}")
        nc.vector.reciprocal(rv[:], var[:])
        rstd = sb.tile([P, 1], f32, tag=tmp_tag + "rs", name=f"ln_rs_{tmp_tag}_{i}")
        nc.scalar.sqrt(rstd[:], rv[:])
        # tmp = (x - mu) * rstd ; out = tmp * w + b
        nc.vector.tensor_scalar(
            out=tmp[:], in0=xi, scalar1=mu[:], scalar2=rstd[:],
            op0=ALU.subtract, op1=ALU.mult)
        nc.vector.tensor_mul(out=tmp[:], in0=tmp[:], in1=w_rep[:])
        nc.vector.tensor_tensor(out=out_view[:, i, :], in0=tmp[:], in1=b_rep[:],
                                op=ALU.add)


def build(stage=9, debug=False):
    nc = bacc.Bacc("TRN2", target_bir_lowering=False, debug=False,
                   num_devices=NCORES)
    _build_graph(nc, stage, debug)
    nc.compile()
    return nc


def _build_graph(nc, stage, debug=False):

    x_ext = nc.dram_tensor("x", [T_OWN, C], f32, kind="ExternalInput").ap()
    wq_ext = nc.dram_tensor("wq", [C, DL], f32, kind="ExternalInput").ap()
    wk_ext = nc.dram_tensor("wk", [C, DL], f32, kind="ExternalInput").ap()
    wv_ext = nc.dram_tensor("wv", [C, DL], f32, kind="ExternalInput").ap()
    wp_ext = nc.dram_tensor("wp", [DL, C], f32, kind="ExternalInput").ap()
    w1_ext = nc.dram_tensor("w1", [C, FF], f32r, kind="ExternalInput").ap()
    w2_ext = nc.dram_tensor("w2", [FF, C], f32, kind="ExternalInput").ap()
    bproj_ext = nc.dram_tensor("bproj", [C], f32, kind="ExternalInput").ap()
    b1_ext = nc.dram_tensor("b1", [FF], f32, kind="ExternalInput").ap()
    b2_ext = nc.dram_tensor("b2", [C], f32, kind="ExternalInput").ap()
    ln1w_ext = nc.dram_tensor("ln1w", [C], f32, kind="ExternalInput").ap()
    ln1b_ext = nc.dram_tensor("ln1b", [C], f32, kind="ExternalInput").ap()
    ln2w_ext = nc.dram_tensor("ln2w", [C], f32, kind="ExternalInput").ap()
    ln2b_ext = nc.dram_tensor("ln2b", [C], f32, kind="ExternalInput").ap()
    out_ext = nc.dram_tensor("out", [T_OWN, C], f32, kind="ExternalOutput").ap()
    dbg = {}
    if debug:
        dbg["mask"] = nc.dram_tensor("dbg_mask", [P, 4, CH], f32,
                                     kind="ExternalOutput").ap()
        dbg["ex"] = nc.dram_tensor("dbg_ex", [P, 2, 2 * CH], f32,
                                   kind="ExternalOutput").ap()
        dbg["aps"] = nc.dram_tensor("dbg_aps", [P, 2 * (D + 1)], f32,
                                    kind="ExternalOutput").ap()

    with tile.TileContext(nc) as tc:
        with (
            tc.tile_pool(name="sb", bufs=1) as sb,
            tc.tile_pool(name="st", bufs=3) as st,    # streaming stages
            tc.tile_pool(name="ps", bufs=1, space="PSUM") as ps,
            tc.tile_pool(name="dram", bufs=1, space="DRAM") as dram,
        ):
            # ---- constants / replicated vectors ----
            id_bf = sb.tile([P, P], bf16)
            make_identity(nc, id_bf[:])
            id_f32 = sb.tile([P, P], f32)
            make_identity(nc, id_f32[:])
            id_fr = sb.tile([P, P], f32r)
            nc.vector.tensor_copy(out=id_fr[:], in_=id_f32[:])

            def rep_pair(ext_a, ext_b, tag, name):
                t = sb.tile([P, 2, C], f32, tag=tag, name=name)
                nc.sync.dma_start(t[:, 0, :], ext_a[None, :].to_broadcast([P, C]))
                nc.sync.dma_start(t[:, 1, :], ext_b[None, :].to_broadcast([P, C]))
                return t[:, 0, :], t[:, 1, :]

            ln1w_r, ln1b_r = rep_pair(ln1w_ext, ln1b_ext, "repA", "ln1_rep")
            bproj_r, b2_r = rep_pair(bproj_ext, b2_ext, "repB", "res_rep")
            b1_sb = sb.tile([P, FMT], f32)
            nc.sync.dma_start(b1_sb[:], b1_ext.rearrange("(m p) -> p m", p=P))

            # causal masks for diagonal blocks: mask_sh[p, hdup, y] =
            # 1 where key (128*sh + p) <= query y, else 0
            masks = []
            for sh in range(2):
                m = sb.tile([P, 2, CH], bf16, name=f"mask{sh}")
                nc.gpsimd.memset(m[:], 1.0)
                nc.gpsimd.affine_select(
                    out=m[:], in_=m[:], compare_op=ALU.is_ge, fill=0.0,
                    base=-128 * sh, pattern=[[0, 2], [1, CH]],
                    channel_multiplier=-1)
                masks.append(m)
            if debug:
                dbgm = sb.tile([P, 4, CH], f32, tag="T32w", name="dbgm")
                for sh in range(2):
                    nc.vector.tensor_copy(out=dbgm[:, 2 * sh:2 * sh + 2, :],
                                          in_=masks[sh][:])
                nc.sync.dma_start(dbg["mask"], dbgm[:])

            # ---- load x, LN1 -> h (bf16) ----
            x_sb = sb.tile([P, IT, C], f32, tag="T16", name="x_sb")
            nc.sync.dma_start(x_sb[:], x_ext.rearrange("(i p) c -> p i c", p=P))
            h = sb.tile([P, IT, C], bf16, tag="T8h", name="h")
            _layer_norm(nc, sb, x_sb, ln1w_r, ln1b_r, h, "ln1")

            # ---- transpose h -> hT_own [P, KT, T_OWN] bf16 ----
            hT_own = sb.tile([P, KT, T_OWN], bf16, tag="T16b", name="hT_own")
            for i in range(IT):
                for ct in range(KT):
                    tp = ps.tile([P, P], bf16, tag="tp", bufs=1,
                                 name=f"tp_h_{i}_{ct}")
                    nc.tensor.transpose(tp[:], h[:, i, ct * P:(ct + 1) * P], id_bf[:])
                    nc.vector.tensor_copy(out=hT_own[:, ct, i * P:(i + 1) * P],
                                          in_=tp[:])

            # ---- AllGather hT ----
            ag_in = dram.tile([C, T_OWN], bf16)
            ag_out = dram.tile([4 * C, T_OWN], bf16)
            nc.sync.dma_start(ag_in.rearrange("(kt kp) t -> kp kt t", kp=P),
                              hT_own[:])
            nc.gpsimd.collective_compute(
                "AllGather", ALU.bypass, ins=[ag_in.opt()], outs=[ag_out.opt()],
                replica_groups=GROUPS)
            hT_all = sb.tile([P, KT, 4, T_OWN], bf16, tag="T32", name="hT_all")
            for r in range(4):
                nc.sync.dma_start(
                    hT_all[:, :, r, :],
                    ag_out[r * C:(r + 1) * C, :].rearrange(
                        "(kt kp) t -> kp kt t", kp=P))

            if stage < 2:
                return
            # ---- cast W slices to bf16 ----
            wqkv_bf = sb.tile([P, 3, KT, DL], bf16, tag="T16c", name="wqkv_bf")
            for wi, ext in enumerate((wq_ext, wk_ext, wv_ext)):
                wst = sb.tile([P, KT, DL], f32, tag="T32w", bufs=1,
                              name=f"w{wi}_st")
                nc.sync.dma_start(wst[:],
                                  ext.rearrange("(kt kp) d -> kp kt d", kp=P))
                nc.vector.tensor_copy(out=wqkv_bf[:, wi], in_=wst[:])
            wq_bf, wk_bf, wv_bf = wqkv_bf[:, 0], wqkv_bf[:, 1], wqkv_bf[:, 2]
            wp_st = sb.tile([P, 2, C], f32, tag="T32w", bufs=1, name="wp_st")
            nc.sync.dma_start(wp_st[:],
                              wp_ext.rearrange("(kt kp) c -> kp kt c", kp=P))
            wp_bf = sb.tile([P, 2, C], bf16, tag="T4p", name="wp_bf")
            nc.vector.tensor_copy(out=wp_bf[:], in_=wp_st[:])

            # ---- QKV ----
            qT = sb.tile([P, 2, T_ALL], bf16, tag="T8q", name="qT")
            kT_lo = sb.tile([P, 2, T_ALL], bf16, tag="T8k", name="kT_lo")
            kT_hi = sb.tile([P, 2, T_ALL], bf16, tag="T8k2", name="kT_hi")
            nc.vector.memset(kT_lo[64:128, :, :], 0.0)
            nc.vector.memset(kT_hi[0:64, :, :], 0.0)
            v_aug = sb.tile([P, QC * 2, NH, D + 1], bf16, tag="T16b", name="v_aug")
            nc.vector.memset(v_aug[:, :, :, D:D + 1], 1.0)

            for wi, w_bf in enumerate((wq_bf, wk_bf)):
                for mt in range(2):
                    for r in range(4):
                        pp = ps.tile([P, T_OWN], f32, tag="big", bufs=3,
                                     name=f"qkv_{wi}_{mt}_{r}")
                        for kt in range(KT):
                            nc.tensor.matmul(
                                pp[:], w_bf[:, kt, mt * P:(mt + 1) * P],
                                hT_all[:, kt, r, :],
                                start=(kt == 0), stop=(kt == KT - 1))
                        if wi == 0:
                            nc.vector.tensor_copy(
                                out=qT[:, mt, r * T_OWN:(r + 1) * T_OWN], in_=pp[:])
                        else:
                            nc.vector.tensor_copy(
                                out=kT_lo[0:64, mt, r * T_OWN:(r + 1) * T_OWN],
                                in_=pp[0:64, :])
                            nc.vector.tensor_copy(
                                out=kT_hi[64:128, mt, r * T_OWN:(r + 1) * T_OWN],
                                in_=pp[64:128, :])
            for stt in range(QC * 2):
                r, i = stt // IT, stt % IT
                pp = ps.tile([P, T_OWN], f32, tag="big", bufs=3,
                             name=f"v_{stt}")
                for kt in range(KT):
                    nc.tensor.matmul(
                        pp[:, :DL],
                        hT_all[:, kt, r, i * P:(i + 1) * P],
                        wv_bf[:, kt, :],
                        start=(kt == 0), stop=(kt == KT - 1))
                nc.vector.tensor_copy(
                    out=v_aug[:, stt, :, 0:D],
                    in_=pp[:, :DL].rearrange("p (h d) -> p h d", d=D))

            if stage < 3:
                return
            # ---- attention ----
            attn_sb = sb.tile([P, QC * 2, DL], bf16, tag="T8h", name="attn_sb")
            for hp in range(2):
                for qc in range(QC):
                    aps = [ps.tile([P, D + 1], f32, tag="attn", bufs=4,
                                   name=f"attn_{hp}_{qc}_{i}")
                           for i in range(4)]
                    for kc in range(qc + 1):
                        for sh in range(2):
                            sc = ps.tile([P, 2 * CH], f32, tag="big", bufs=3,
                                         name=f"sc_{hp}_{qc}_{kc}_{sh}")
                            for hl in range(2):
                                kTv = kT_lo if hl == 0 else kT_hi
                                nc.tensor.matmul(
                                    sc[:, hl * CH:(hl + 1) * CH],
                                    kTv[:, hp,
                                        kc * CH + sh * P: kc * CH + (sh + 1) * P],
                                    qT[:, hp, qc * CH:(qc + 1) * CH],
                                    start=True, stop=True)
                            ex = st.tile([P, 2 * CH], bf16, tag="expT", bufs=3,
                                         name=f"ex_{hp}_{qc}_{kc}_{sh}")
                            nc.scalar.activation(ex[:], sc[:], ACT_F.Exp,
                                                 bias=0.0, scale=SCALE)
                            if kc == qc:
                                nc.vector.tensor_tensor(
                                    out=ex.rearrange("p (a y) -> p a y", y=CH),
                                    in0=ex.rearrange("p (a y) -> p a y", y=CH),
                                    in1=masks[sh][:], op=ALU.mult)
                            if debug and hp == 0 and qc == 0:
                                dbge = sb.tile([P, 2, 2 * CH], f32, tag="T32w",
                                               name=f"dbge_{sh}")
                                nc.vector.tensor_copy(out=dbge[:, sh, :], in_=ex[:])
                                if sh == 1:
                                    nc.sync.dma_start(dbg["ex"], dbge[:])
                            for hl in range(2):
                                for ti in range(2):
                                    nc.tensor.matmul(
                                        aps[hl * 2 + ti][:],
                                        ex[:, hl * CH + ti * P: hl * CH + (ti + 1) * P],
                                        v_aug[:, 2 * kc + sh, 2 * hp + hl, :],
                                        start=(kc == 0 and sh == 0),
                                        stop=(kc == qc and sh == 1))
                    if debug and hp == 0 and qc == 0:
                        dbga = sb.tile([P, 2 * (D + 1)], f32, name="dbga")
                        nc.vector.tensor_copy(out=dbga[:, 0:D + 1], in_=aps[0][:])
                        nc.vector.tensor_copy(out=dbga[:, D + 1:], in_=aps[1][:])
                        nc.sync.dma_start(dbg["aps"], dbga[:])
                    for hl in range(2):
                        for ti in range(2):
                            a = aps[hl * 2 + ti]
                            rd = st.tile([P, 1], f32, tag="rd", bufs=4,
                                         name=f"rd_{hp}_{qc}_{hl}_{ti}")
                            nc.vector.reciprocal(rd[:], a[:, D:D + 1])
                            nc.vector.tensor_scalar(
                                out=attn_sb[:, 2 * qc + ti,
                                            (2 * hp + hl) * D:(2 * hp + hl + 1) * D],
                                in0=a[:, 0:D],
                                scalar1=rd[:], scalar2=None, op0=ALU.mult)

            if stage < 4:
                return
            # ---- transpose attn -> attnT [P, 2, T_ALL] bf16 ----
            attnT = sb.tile([P, 2, T_ALL], bf16, tag="T8q", name="attnT")
            for tt in range(QC * 2):
                for ct in range(2):
                    tp = ps.tile([P, P], bf16, tag="tp", bufs=1,
                                 name=f"tp_a_{tt}_{ct}")
                    nc.tensor.transpose(tp[:], attn_sb[:, tt, ct * P:(ct + 1) * P],
                                        id_bf[:])
                    nc.vector.tensor_copy(out=attnT[:, ct, tt * P:(tt + 1) * P],
                                          in_=tp[:])

            # ---- proj partial -> rs_dram ----
            rs_in = dram.tile([T_ALL, C], bf16)
            rs_out = dram.tile([T_OWN, C], bf16)
            for mt in range(QC * 2):
                ob = st.tile([P, C], bf16, tag="projev", bufs=2, name=f"projev_{mt}")
                for n in range(2):
                    pp = ps.tile([P, 512], f32, tag="big", bufs=3,
                                 name=f"proj_{mt}_{n}")
                    for kt2 in range(2):
                        nc.tensor.matmul(
                            pp[:], attnT[:, kt2, mt * P:(mt + 1) * P],
                            wp_bf[:, kt2, n * 512:(n + 1) * 512],
                            start=(kt2 == 0), stop=(kt2 == 1))
                    nc.vector.tensor_copy(out=ob[:, n * 512:(n + 1) * 512],
                                          in_=pp[:])
                nc.sync.dma_start(rs_in[mt * P:(mt + 1) * P, :], ob[:])
            nc.gpsimd.collective_compute(
                "ReduceScatter", ALU.add, ins=[rs_in.opt()], outs=[rs_out.opt()],
                replica_groups=GROUPS)

            if stage < 5:
                return
            # ---- residual 1: out1 = x + rs + bproj ----
            rs_sb = sb.tile([P, IT, C], bf16)
            nc.sync.dma_start(rs_sb[:], rs_out.rearrange("(i p) c -> p i c", p=P))
            out1 = sb.tile([P, IT, C], f32, tag="T16c", name="out1")
            for i in range(IT):
                nc.vector.tensor_tensor(out=out1[:, i, :], in0=x_sb[:, i, :],
                                        in1=rs_sb[:, i, :], op=ALU.add)
                nc.vector.tensor_tensor(out=out1[:, i, :], in0=out1[:, i, :],
                                        in1=bproj_r[:], op=ALU.add)

            # ---- LN2 -> h2 (f32r) ----
            ln2w_r, ln2b_r = rep_pair(ln2w_ext, ln2b_ext, "repA", "ln2_rep")
            h2 = sb.tile([P, IT, C], f32r, tag="T16", name="h2")
            _layer_norm(nc, sb, out1, ln2w_r, ln2b_r, h2, "ln2")

            # ---- transpose h2 -> h2T [P, KT, T_OWN] f32r ----
            h2T = sb.tile([P, KT, T_OWN], f32r, tag="T16b", name="h2T")
            for i in range(IT):
                for ct in range(KT):
                    tp = ps.tile([P, P], f32r, tag="tp", bufs=1,
                                 name=f"tp_h2_{i}_{ct}")
                    nc.tensor.transpose(tp[:], h2[:, i, ct * P:(ct + 1) * P],
                                        id_fr[:])
                    nc.vector.tensor_copy(out=h2T[:, ct, i * P:(i + 1) * P],
                                          in_=tp[:])

            if stage < 6:
                return
            # ---- FFN1 (f32r): ff1T[m, t] = relu(W1.T h2T + b1) ----
            ff1T = sb.tile([P, FMT, T_OWN], bf16, tag="T32", name="ff1T")
            for mt in range(FMT):
                w1s = st.tile([P, KT, P], f32r, tag="w1st", bufs=2, name=f"w1st_{mt}")
                nc.sync.dma_start(
                    w1s[:],
                    w1_ext[:, mt * P:(mt + 1) * P].rearrange(
                        "(kt kp) m -> kp kt m", kp=P))
                pp = ps.tile([P, T_OWN], f32, tag="big", bufs=3,
                             name=f"ff1_{mt}")
                for kt in range(KT):
                    nc.tensor.matmul(pp[:], w1s[:, kt, :], h2T[:, kt, :],
                                     start=(kt == 0), stop=(kt == KT - 1))
                nc.scalar.activation(ff1T[:, mt, :], pp[:], ACT_F.Relu,
                                     bias=b1_sb[:, mt:mt + 1])

            # ---- FFN2 (bf16): two n-half passes, W2 streamed+cast per pass ----
            for n in range(2):
                w2h = sb.tile([P, FMT, 512], bf16, tag="T32w", name=f"w2h_{n}")
                for kt in range(FMT):
                    w2s = st.tile([P, 512], f32, tag="w2st", bufs=2,
                                  name=f"w2st_{n}_{kt}")
                    nc.sync.dma_start(
                        w2s[:], w2_ext[kt * P:(kt + 1) * P,
                                       n * 512:(n + 1) * 512])
                    nc.gpsimd.tensor_copy(out=w2h[:, kt, :], in_=w2s[:])
                for m in range(IT):
                    pp = ps.tile([P, 512], f32, tag="big", bufs=3,
                                 name=f"ff2_{m}_{n}")
                    for kt in range(FMT):
                        nc.tensor.matmul(
                            pp[:], ff1T[:, kt, m * P:(m + 1) * P],
                            w2h[:, kt, :],
                            start=(kt == 0), stop=(kt == FMT - 1))
                    ob = st.tile([P, 512], f32, tag="outev", bufs=2,
                                 name=f"outev_{m}_{n}")
                    nc.vector.tensor_tensor(
                        out=ob[:], in0=pp[:],
                        in1=out1[:, m, n * 512:(n + 1) * 512], op=ALU.add)
                    nc.vector.tensor_tensor(
                        out=ob[:], in0=ob[:],
                        in1=b2_r[:, n * 512:(n + 1) * 512], op=ALU.add)
                    nc.sync.dma_start(
                        out_ext[m * P:(m + 1) * P, n * 512:(n + 1) * 512],
                        ob[:])


_NC_CACHE = None


def _get_nc():
    global _NC_CACHE
    if _NC_CACHE is None:
        _NC_CACHE = build()
    return _NC_CACHE


def shard_inputs(x, Wq, Wk, Wv, Wproj, bproj, W1, b1, W2, b2,
                 ln1_w, ln1_b, ln2_w, ln2_b):
    in_maps = []
    for c in range(NCORES):
        b, j = c // 4, c % 4
        hs = slice(DL * j, DL * (j + 1))
        in_maps.append({
            "x": np.ascontiguousarray(x[b, T_OWN * j:T_OWN * (j + 1)], np.float32),
            "wq": np.ascontiguousarray(Wq[:, hs], np.float32),
            "wk": np.ascontiguousarray(Wk[:, hs], np.float32),
            "wv": np.ascontiguousarray(Wv[:, hs], np.float32),
            "wp": np.ascontiguousarray(Wproj[hs, :], np.float32),
            "w1": np.ascontiguousarray(W1, np.float32),
            "w2": np.ascontiguousarray(W2, np.float32),
            "bproj": np.ascontiguousarray(bproj, np.float32),
            "b1": np.ascontiguousarray(b1, np.float32),
            "b2": np.ascontiguousarray(b2, np.float32),
            "ln1w": np.ascontiguousarray(ln1_w, np.float32),
            "ln1b": np.ascontiguousarray(ln1_b, np.float32),
            "ln2w": np.ascontiguousarray(ln2_w, np.float32),
            "ln2b": np.ascontiguousarray(ln2_b, np.float32),
        })
    return in_maps


def assemble(results):
    out = np.empty((2, T_ALL, C), np.float32)
    for c in range(NCORES):
        b, j = c // 4, c % 4
        out[b, T_OWN * j:T_OWN * (j + 1)] = results[c]["out"]
    return out


def kernel(**inputs):
    nc = _get_nc()
    in_maps = shard_inputs(**{k: np.asarray(v) for k, v in inputs.items()})
    res = run_bass_kernel_spmd(nc, in_maps, list(range(NCORES)))
    return assemble(res.results)


# revision 15
# speedup vs baseline: 1.0460x; 1.0460x over previous
"""Transformer block (pre-LN attention + FFN) on 8 TRN2 NeuronCores.

Sharding: batch x head tensor-parallel for attention, sequence-parallel for
LN/FFN/residual. Core c (b = c//4, j = c%4):
  - owns token shard [512j : 512j+512) of batch b for LN1/LN2/FFN/residual
  - owns heads [4j : 4j+4) of batch b for attention (all 2048 tokens)
Collectives (per-batch groups [[0..3],[4..7]]):
  - AllGather of transposed LN1 output hT (bf16) so every core sees all tokens
  - ReduceScatter (add) of the attention projection partial sums back to
    token shards.
All programs are identical across cores (SPMD); only input data differs.

Matmul dtypes: float32r (full-rate fp32, no cast needed) for the
weight-side matmuls fed by f32 DRAM (FFN1), bf16 for attention internals +
QKV/proj/FFN2 (operands produced on-chip, cast free on evacuation).
"""

import numpy as np

import concourse.bass as bass
import concourse.mybir as mybir
import concourse.tile as tile
from concourse import bacc
from concourse.bass_utils import run_bass_kernel_spmd
from concourse.masks import make_identity

P = 128
C = 1024          # n_embd
KT = C // P       # 8 c-tiles
T_OWN = 512       # tokens per core (sequence shard)
IT = T_OWN // P   # 4 own t-tiles
T_ALL = 2048      # tokens per batch
NH = 4            # heads per core
D = 64            # head dim
DL = NH * D       # 256 local head features
FF = 4096
FMT = FF // P     # 32 ffn m-tiles
CH = 256          # attention chunk
QC = T_ALL // CH  # 8 chunks
EPS = 1e-5
SCALE = 1.0 / 32.0  # C ** -0.5
GROUPS = [[0, 1, 2, 3], [4, 5, 6, 7]]
NCORES = 8

f32 = mybir.dt.float32
f32r = mybir.dt.float32r
bf16 = mybir.dt.bfloat16
AX = mybir.AxisListType
ALU = mybir.AluOpType
ACT_F = mybir.ActivationFunctionType


def _layer_norm(nc, sb, st, x_view, w_rep, b_rep, out_view, tmp_tag):
    """Token-major LN over free axis. x_view/out_view: [P, IT, C].
    Stats vectorized across the IT tiles; sum-of-squares via ACT Square with
    fused row-accumulate."""
    ssum = sb.tile([P, IT], f32, tag=tmp_tag + "s", name=f"ln_s_{tmp_tag}")
    sqs = sb.tile([P, IT], f32, tag=tmp_tag + "q", name=f"ln_q_{tmp_tag}")
    for i in range(IT):
        nc.vector.tensor_reduce(ssum[:, i:i + 1], x_view[:, i, :], AX.X, ALU.add)
        sq = st.tile([P, C], f32, tag="outev", bufs=2, name=f"ln_sq_{tmp_tag}_{i}")
        nc.scalar.activation(sq[:], x_view[:, i, :], ACT_F.Square,
                             accum_out=sqs[:, i:i + 1])
    mu = sb.tile([P, IT], f32, tag=tmp_tag + "mu", name=f"ln_mu_{tmp_tag}")
    nc.vector.tensor_scalar_mul(mu[:], ssum[:], 1.0 / C)
    var = sb.tile([P, IT], f32, tag=tmp_tag + "v", name=f"ln_v_{tmp_tag}")
    nc.vector.tensor_scalar_mul(var[:], sqs[:], 1.0 / C)
    musq = sb.tile([P, IT], f32, tag=tmp_tag + "m2", name=f"ln_m2_{tmp_tag}")
    nc.vector.tensor_mul(out=musq[:], in0=mu[:], in1=mu[:])
    nc.vector.tensor_sub(out=var[:], in0=var[:], in1=musq[:])
    nc.vector.tensor_scalar_add(var[:], var[:], EPS)
    rv = sb.tile([P, IT], f32, tag=tmp_tag + "rv", name=f"ln_rv_{tmp_tag}")
    nc.vector.reciprocal(rv[:], var[:])
    rstd = sb.tile([P, IT], f32, tag=tmp_tag + "rs", name=f"ln_rs_{tmp_tag}")
    nc.scalar.sqrt(rstd[:], rv[:])
    for i in range(IT):
        tmp = st.tile([P, C], f32, tag="outev", bufs=2,
                      name=f"ln_tmp_{tmp_tag}_{i}")
        nc.vector.tensor_scalar(
            out=tmp[:], in0=x_view[:, i, :], scalar1=mu[:, i:i + 1],
            scalar2=rstd[:, i:i + 1], op0=ALU.subtract, op1=ALU.mult)
        nc.vector.tensor_mul(out=tmp[:], in0=tmp[:], in1=w_rep[:])
        nc.vector.tensor_tensor(out=out_view[:, i, :], in0=tmp[:], in1=b_rep[:],
                                op=ALU.add)


def build(stage=9, debug=False):
    nc = bacc.Bacc("TRN2", target_bir_lowering=False, debug=False,
                   num_devices=NCORES)
    _build_graph(nc, stage, debug)
    nc.compile()
    return nc


def _build_graph(nc, stage, debug=False):

    x_ext = nc.dram_tensor("x", [T_OWN, C], f32, kind="ExternalInput").ap()
    wq_ext = nc.dram_tensor("wq", [C, DL], f32, kind="ExternalInput").ap()
    wk_ext = nc.dram_tensor("wk", [C, DL], f32, kind="ExternalInput").ap()
    wv_ext = nc.dram_tensor("wv", [C, DL], f32, kind="ExternalInput").ap()
    wp_ext = nc.dram_tensor("wp", [DL, C], f32, kind="ExternalInput").ap()
    w1_ext = nc.dram_tensor("w1", [C, FF], f32r, kind="ExternalInput").ap()
    w2_ext = nc.dram_tensor("w2", [FF, C], f32, kind="ExternalInput").ap()
    bproj_ext = nc.dram_tensor("bproj", [C], f32, kind="ExternalInput").ap()
    b1_ext = nc.dram_tensor("b1", [FF], f32, kind="ExternalInput").ap()
    b2_ext = nc.dram_tensor("b2", [C], f32, kind="ExternalInput").ap()
    ln1w_ext = nc.dram_tensor("ln1w", [C], f32, kind="ExternalInput").ap()
    ln1b_ext = nc.dram_tensor("ln1b", [C], f32, kind="ExternalInput").ap()
    ln2w_ext = nc.dram_tensor("ln2w", [C], f32, kind="ExternalInput").ap()
    ln2b_ext = nc.dram_tensor("ln2b", [C], f32, kind="ExternalInput").ap()
    out_ext = nc.dram_tensor("out", [T_OWN, C], f32, kind="ExternalOutput").ap()
    dbg = {}
    if debug:
        dbg["mask"] = nc.dram_tensor("dbg_mask", [P, 4, CH], f32,
                                     kind="ExternalOutput").ap()
        dbg["ex"] = nc.dram_tensor("dbg_ex", [P, 2, 2 * CH], f32,
                                   kind="ExternalOutput").ap()
        dbg["aps"] = nc.dram_tensor("dbg_aps", [P, 2 * (D + 1)], f32,
                                    kind="ExternalOutput").ap()

    with tile.TileContext(nc) as tc:
        with (
            tc.tile_pool(name="sb", bufs=1) as sb,
            tc.tile_pool(name="st", bufs=3) as st,    # streaming stages
            tc.tile_pool(name="ps", bufs=1, space="PSUM") as ps,
            tc.tile_pool(name="dram", bufs=1, space="DRAM") as dram,
        ):
            # ---- constants / replicated vectors ----
            id_bf = sb.tile([P, P], bf16)
            make_identity(nc, id_bf[:])
            id_f32 = sb.tile([P, P], f32)
            make_identity(nc, id_f32[:])
            id_fr = sb.tile([P, P], f32r)
            nc.vector.tensor_copy(out=id_fr[:], in_=id_f32[:])

            def rep_pair(ext_a, ext_b, tag, name):
                t = sb.tile([P, 2, C], f32, tag=tag, name=name)
                nc.sync.dma_start(t[:, 0, :], ext_a[None, :].to_broadcast([P, C]))
                nc.sync.dma_start(t[:, 1, :], ext_b[None, :].to_broadcast([P, C]))
                return t[:, 0, :], t[:, 1, :]

            ln1w_r, ln1b_r = rep_pair(ln1w_ext, ln1b_ext, "repA", "ln1_rep")
            bproj_r, b2_r = rep_pair(bproj_ext, b2_ext, "repB", "res_rep")
            b1_sb = sb.tile([P, FMT], f32)
            nc.sync.dma_start(b1_sb[:], b1_ext.rearrange("(m p) -> p m", p=P))

            # causal masks for diagonal blocks: mask_sh[p, hdup, y] =
            # 1 where key (128*sh + p) <= query y, else 0
            masks = []
            for sh in range(2):
                m = sb.tile([P, 2, CH], bf16, name=f"mask{sh}")
                nc.gpsimd.memset(m[:], 1.0)
                nc.gpsimd.affine_select(
                    out=m[:], in_=m[:], compare_op=ALU.is_ge, fill=0.0,
                    base=-128 * sh, pattern=[[0, 2], [1, CH]],
                    channel_multiplier=-1)
                masks.append(m)
            if debug:
                dbgm = sb.tile([P, 4, CH], f32, tag="T32w", name="dbgm")
                for sh in range(2):
                    nc.vector.tensor_copy(out=dbgm[:, 2 * sh:2 * sh + 2, :],
                                          in_=masks[sh][:])
                nc.sync.dma_start(dbg["mask"], dbgm[:])

            # ---- load x, LN1 -> h (bf16) ----
            x_sb = sb.tile([P, IT, C], f32, tag="T16", name="x_sb")
            for i in range(IT):
                nc.sync.dma_start(x_sb[:, i, :], x_ext[i * P:(i + 1) * P, :])
            h = sb.tile([P, IT, C], bf16, tag="T8h", name="h")
            _layer_norm(nc, sb, st, x_sb, ln1w_r, ln1b_r, h, "ln1")
            for i in range(IT):
                nc.vector.tensor_tensor(out=x_sb[:, i, :], in0=x_sb[:, i, :],
                                        in1=bproj_r[:], op=ALU.add)

            # ---- transpose h -> hT_own [P, KT, T_OWN] bf16 ----
            hT_own = sb.tile([P, KT, T_OWN], bf16, tag="T16b", name="hT_own")
            ag_in = dram.tile([C, T_OWN], bf16)
            ag_out = dram.tile([4 * C, T_OWN], bf16)
            for ct in range(KT):
                for i in range(IT):
                    tp = ps.tile([P, P], bf16, tag="tp", bufs=2,
                                 name=f"tp_h_{i}_{ct}")
                    nc.tensor.transpose(tp[:], h[:, i, ct * P:(ct + 1) * P], id_bf[:])
                    nc.vector.tensor_copy(out=hT_own[:, ct, i * P:(i + 1) * P],
                                          in_=tp[:])
                nc.sync.dma_start(ag_in[ct * P:(ct + 1) * P, :], hT_own[:, ct, :])

            # ---- AllGather hT ----
            nc.gpsimd.collective_compute(
                "AllGather", ALU.bypass, ins=[ag_in.opt()], outs=[ag_out.opt()],
                replica_groups=GROUPS)
            hT_all = sb.tile([P, KT, 4, T_OWN], bf16, tag="T32", name="hT_all")
            for r in range(4):
                nc.sync.dma_start(
                    hT_all[:, :, r, :],
                    ag_out[r * C:(r + 1) * C, :].rearrange(
                        "(kt kp) t -> kp kt t", kp=P))

            if stage < 2:
                return
            # ---- cast W slices to bf16 ----
            wqkv_bf = sb.tile([P, 3, KT, DL], bf16, tag="T16c", name="wqkv_bf")
            for wi, ext in enumerate((wq_ext, wk_ext, wv_ext)):
                wst = sb.tile([P, KT, DL], f32, tag="T32w", bufs=1,
                              name=f"w{wi}_st")
                nc.sync.dma_start(wst[:],
                                  ext.rearrange("(kt kp) d -> kp kt d", kp=P))
                nc.vector.tensor_copy(out=wqkv_bf[:, wi], in_=wst[:])
            wq_bf, wk_bf, wv_bf = wqkv_bf[:, 0], wqkv_bf[:, 1], wqkv_bf[:, 2]
            wp_st = sb.tile([P, 2, C], f32, tag="T32w", bufs=1, name="wp_st")
            nc.sync.dma_start(wp_st[:],
                              wp_ext.rearrange("(kt kp) c -> kp kt c", kp=P))
            wp_bf = sb.tile([P, 2, C], bf16, tag="T4p", name="wp_bf")
            nc.vector.tensor_copy(out=wp_bf[:], in_=wp_st[:])

            # ---- QKV ----
            qT = sb.tile([P, 2, T_ALL], bf16, tag="T8q", name="qT")
            kT_lo = sb.tile([P, 2, T_ALL], bf16, tag="T8k", name="kT_lo")
            kT_hi = sb.tile([P, 2, T_ALL], bf16, tag="T8k2", name="kT_hi")
            nc.vector.memset(kT_lo[64:128, :, :], 0.0)
            nc.vector.memset(kT_hi[0:64, :, :], 0.0)
            v_aug = sb.tile([P, QC * 2, NH, D + 1], bf16, tag="T16b", name="v_aug")
            nc.vector.memset(v_aug[:, :, :, D:D + 1], 1.0)

            for wi, w_bf in enumerate((wq_bf, wk_bf)):
                for mt in range(2):
                    for r in range(4):
                        pp = ps.tile([P, T_OWN], f32, tag="big", bufs=2,
                                     name=f"qkv_{wi}_{mt}_{r}")
                        for kt in range(KT):
                            nc.tensor.matmul(
                                pp[:], w_bf[:, kt, mt * P:(mt + 1) * P],
                                hT_all[:, kt, r, :],
                                start=(kt == 0), stop=(kt == KT - 1))
                        if wi == 0:
                            nc.vector.tensor_copy(
                                out=qT[:, mt, r * T_OWN:(r + 1) * T_OWN], in_=pp[:])
                        else:
                            nc.vector.tensor_copy(
                                out=kT_lo[0:64, mt, r * T_OWN:(r + 1) * T_OWN],
                                in_=pp[0:64, :])
                            nc.vector.tensor_copy(
                                out=kT_hi[64:128, mt, r * T_OWN:(r + 1) * T_OWN],
                                in_=pp[64:128, :])
            for stt in range(QC * 2):
                r, i = stt // IT, stt % IT
                pp = ps.tile([P, T_OWN], f32, tag="big", bufs=2,
                             name=f"v_{stt}")
                for kt in range(KT):
                    nc.tensor.matmul(
                        pp[:, :DL],
                        hT_all[:, kt, r, i * P:(i + 1) * P],
                        wv_bf[:, kt, :],
                        start=(kt == 0), stop=(kt == KT - 1))
                nc.vector.tensor_copy(
                    out=v_aug[:, stt, :, 0:D],
                    in_=pp[:, :DL].rearrange("p (h d) -> p h d", d=D))

            if stage < 3:
                return
            # ---- attention ----
            attn_sb = sb.tile([P, QC * 2, DL], bf16, tag="T8h", name="attn_sb")
            for hp in range(2):
                for qc in range(QC):
                    aps = [ps.tile([P, D + 1], f32, tag="attn", bufs=4,
                                   name=f"attn_{hp}_{qc}_{i}")
                           for i in range(4)]
                    for kc in range(qc + 1):
                        for sh in range(2):
                            sc = ps.tile([P, 2 * CH], f32, tag="big", bufs=2,
                                         name=f"sc_{hp}_{qc}_{kc}_{sh}")
                            for hl in range(2):
                                kTv = kT_lo if hl == 0 else kT_hi
                                nc.tensor.matmul(
                                    sc[:, hl * CH:(hl + 1) * CH],
                                    kTv[:, hp,
                                        kc * CH + sh * P: kc * CH + (sh + 1) * P],
                                    qT[:, hp, qc * CH:(qc + 1) * CH],
                                    start=True, stop=True)
                            ex = st.tile([P, 2 * CH], bf16, tag="expT", bufs=3,
                                         name=f"ex_{hp}_{qc}_{kc}_{sh}")
                            nc.scalar.activation(ex[:], sc[:], ACT_F.Exp,
                                                 bias=0.0, scale=SCALE)
                            if kc == qc:
                                nc.vector.tensor_tensor(
                                    out=ex.rearrange("p (a y) -> p a y", y=CH),
                                    in0=ex.rearrange("p (a y) -> p a y", y=CH),
                                    in1=masks[sh][:], op=ALU.mult)
                            if debug and hp == 0 and qc == 0:
                                dbge = sb.tile([P, 2, 2 * CH], f32, tag="T32w",
                                               name=f"dbge_{sh}")
                                nc.vector.tensor_copy(out=dbge[:, sh, :], in_=ex[:])
                                if sh == 1:
                                    nc.sync.dma_start(dbg["ex"], dbge[:])
                            for hl in range(2):
                                for ti in range(2):
                                    nc.tensor.matmul(
                                        aps[hl * 2 + ti][:],
                                        ex[:, hl * CH + ti * P: hl * CH + (ti + 1) * P],
                                        v_aug[:, 2 * kc + sh, 2 * hp + hl, :],
                                        start=(kc == 0 and sh == 0),
                                        stop=(kc == qc and sh == 1))
                    if debug and hp == 0 and qc == 0:
                        dbga = sb.tile([P, 2 * (D + 1)], f32, name="dbga")
                        nc.vector.tensor_copy(out=dbga[:, 0:D + 1], in_=aps[0][:])
                        nc.vector.tensor_copy(out=dbga[:, D + 1:], in_=aps[1][:])
                        nc.sync.dma_start(dbg["aps"], dbga[:])
                    for hl in range(2):
                        for ti in range(2):
                            a = aps[hl * 2 + ti]
                            rd = st.tile([P, 1], f32, tag="rd", bufs=4,
                                         name=f"rd_{hp}_{qc}_{hl}_{ti}")
                            nc.vector.reciprocal(rd[:], a[:, D:D + 1])
                            nc.vector.tensor_scalar(
                                out=attn_sb[:, 2 * qc + ti,
                                            (2 * hp + hl) * D:(2 * hp + hl + 1) * D],
                                in0=a[:, 0:D],
                                scalar1=rd[:], scalar2=None, op0=ALU.mult)

            if stage < 4:
                return
            # ---- transpose attn -> attnT [P, 2, T_ALL] bf16 ----
            attnT = sb.tile([P, 2, T_ALL], bf16, tag="T8q", name="attnT")
            for tt in range(QC * 2):
                for ct in range(2):
                    tp = ps.tile([P, P], bf16, tag="tp", bufs=2,
                                 name=f"tp_a_{tt}_{ct}")
                    nc.tensor.transpose(tp[:], attn_sb[:, tt, ct * P:(ct + 1) * P],
                                        id_bf[:])
                    nc.vector.tensor_copy(out=attnT[:, ct, tt * P:(tt + 1) * P],
                                          in_=tp[:])

            # ---- proj partial -> rs_dram ----
            rs_in = dram.tile([T_ALL, C], bf16)
            rs_out = dram.tile([T_OWN, C], bf16)
            for mt in range(QC * 2):
                ob = st.tile([P, C], bf16, tag="projev", bufs=2, name=f"projev_{mt}")
                for n in range(2):
                    pp = ps.tile([P, 512], f32, tag="big", bufs=2,
                                 name=f"proj_{mt}_{n}")
                    for kt2 in range(2):
                        nc.tensor.matmul(
                            pp[:], attnT[:, kt2, mt * P:(mt + 1) * P],
                            wp_bf[:, kt2, n * 512:(n + 1) * 512],
                            start=(kt2 == 0), stop=(kt2 == 1))
                    nc.vector.tensor_copy(out=ob[:, n * 512:(n + 1) * 512],
                                          in_=pp[:])
                nc.sync.dma_start(rs_in[mt * P:(mt + 1) * P, :], ob[:])
            nc.gpsimd.collective_compute(
                "ReduceScatter", ALU.add, ins=[rs_in.opt()], outs=[rs_out.opt()],
                replica_groups=GROUPS)

            if stage < 5:
                return
            # ---- residual 1: out1 = x + rs + bproj ----
            rs_sb = sb.tile([P, IT, C], bf16)
            nc.sync.dma_start(rs_sb[:], rs_out.rearrange("(i p) c -> p i c", p=P))
            out1 = sb.tile([P, IT, C], f32, tag="T16c", name="out1")
            for i in range(IT):
                nc.vector.tensor_tensor(out=out1[:, i, :], in0=x_sb[:, i, :],
                                        in1=rs_sb[:, i, :], op=ALU.add)

            # ---- LN2 -> h2 (f32r) ----
            ln2w_r, ln2b_r = rep_pair(ln2w_ext, ln2b_ext, "repA", "ln2_rep")
            h2 = sb.tile([P, IT, C], f32r, tag="T16", name="h2")
            _layer_norm(nc, sb, st, out1, ln2w_r, ln2b_r, h2, "ln2")

            # ---- transpose h2 -> h2T [P, KT, T_OWN] f32r ----
            h2T = sb.tile([P, KT, T_OWN], f32r, tag="T16b", name="h2T")
            for i in range(IT):
                for ct in range(KT):
                    tp = ps.tile([P, P], f32r, tag="tp", bufs=2,
                                 name=f"tp_h2_{i}_{ct}")
                    nc.tensor.transpose(tp[:], h2[:, i, ct * P:(ct + 1) * P],
                                        id_fr[:])
                    nc.vector.tensor_copy(out=h2T[:, ct, i * P:(i + 1) * P],
                                          in_=tp[:])

            if stage < 6:
                return
            # ---- FFN1 (f32r): ff1T[m, t] = relu(W1.T h2T + b1) ----
            ff1T = sb.tile([P, FMT, T_OWN], bf16, tag="T32", name="ff1T")
            for mt in range(FMT):
                w1s = st.tile([P, KT, P], f32r, tag="w1st", bufs=3, name=f"w1st_{mt}")
                nc.sync.dma_start(
                    w1s[:],
                    w1_ext[:, mt * P:(mt + 1) * P].rearrange(
                        "(kt kp) m -> kp kt m", kp=P))
                pp = ps.tile([P, T_OWN], f32, tag="big", bufs=2,
                             name=f"ff1_{mt}")
                for kt in range(KT):
                    nc.tensor.matmul(pp[:], w1s[:, kt, :], h2T[:, kt, :],
                                     start=(kt == 0), stop=(kt == KT - 1))
                nc.scalar.activation(ff1T[:, mt, :], pp[:], ACT_F.Relu,
                                     bias=b1_sb[:, mt:mt + 1])

            # ---- FFN2 (bf16): two n-half passes, W2 streamed+cast per pass ----
            for n in range(2):
                w2h = sb.tile([P, FMT, 512], bf16, tag="T32w", name=f"w2h_{n}")
                for kt in range(FMT):
                    w2s = st.tile([P, 512], f32, tag="w2st", bufs=2,
                                  name=f"w2st_{n}_{kt}")
                    nc.sync.dma_start(
                        w2s[:], w2_ext[kt * P:(kt + 1) * P,
                                       n * 512:(n + 1) * 512])
                    nc.gpsimd.tensor_copy(out=w2h[:, kt, :], in_=w2s[:])
                for m in range(IT):
                    pp = ps.tile([P, 512], f32, tag="big", bufs=2,
                                 name=f"ff2_{m}_{n}")
                    for kt in range(FMT):
                        nc.tensor.matmul(
                            pp[:], ff1T[:, kt, m * P:(m + 1) * P],
                            w2h[:, kt, :],
                            start=(kt == 0), stop=(kt == FMT - 1))
                    ob = st.tile([P, 512], f32, tag="outev", bufs=2,
                                 name=f"outev_{m}_{n}")
                    nc.vector.tensor_tensor(
                        out=ob[:], in0=pp[:],
                        in1=out1[:, m, n * 512:(n + 1) * 512], op=ALU.add)
                    nc.vector.tensor_tensor(
                        out=ob[:], in0=ob[:],
                        in1=b2_r[:, n * 512:(n + 1) * 512], op=ALU.add)
                    nc.sync.dma_start(
                        out_ext[m * P:(m + 1) * P, n * 512:(n + 1) * 512],
                        ob[:])


_NC_CACHE = None


def _get_nc():
    global _NC_CACHE
    if _NC_CACHE is None:
        _NC_CACHE = build()
    return _NC_CACHE


def shard_inputs(x, Wq, Wk, Wv, Wproj, bproj, W1, b1, W2, b2,
                 ln1_w, ln1_b, ln2_w, ln2_b):
    in_maps = []
    for c in range(NCORES):
        b, j = c // 4, c % 4
        hs = slice(DL * j, DL * (j + 1))
        in_maps.append({
            "x": np.ascontiguousarray(x[b, T_OWN * j:T_OWN * (j + 1)], np.float32),
            "wq": np.ascontiguousarray(Wq[:, hs], np.float32),
            "wk": np.ascontiguousarray(Wk[:, hs], np.float32),
            "wv": np.ascontiguousarray(Wv[:, hs], np.float32),
            "wp": np.ascontiguousarray(Wproj[hs, :], np.float32),
            "w1": np.ascontiguousarray(W1, np.float32),
            "w2": np.ascontiguousarray(W2, np.float32),
            "bproj": np.ascontiguousarray(bproj, np.float32),
            "b1": np.ascontiguousarray(b1, np.float32),
            "b2": np.ascontiguousarray(b2, np.float32),
            "ln1w": np.ascontiguousarray(ln1_w, np.float32),
            "ln1b": np.ascontiguousarray(ln1_b, np.float32),
            "ln2w": np.ascontiguousarray(ln2_w, np.float32),
            "ln2b": np.ascontiguousarray(ln2_b, np.float32),
        })
    return in_maps


def assemble(results):
    out = np.empty((2, T_ALL, C), np.float32)
    for c in range(NCORES):
        b, j = c // 4, c % 4
        out[b, T_OWN * j:T_OWN * (j + 1)] = results[c]["out"]
    return out


def kernel(**inputs):
    nc = _get_nc()
    in_maps = shard_inputs(**{k: np.asarray(v) for k, v in inputs.items()})
    res = run_bass_kernel_spmd(nc, in_maps, list(range(NCORES)))
    return assemble(res.results)
